# revision 6
# baseline (speedup 1.0000x reference)
"""Corner-detection (structure-tensor min-eigenvalue + edge magnitude)
Bass/Tile kernel for Trainium2, sharded over 8 NeuronCores by image rows.

Pipeline per core (512 image rows), per 128-row tile (5 overlapping
tiles, step 97), per half-width stripe (2 stripes of 2048 cols):
  PE : vs = vertical-smooth(gray), vi = (3/16)*vertical-interp(gray)
       fused with the grayscale channel mix (banded lhsT per channel,
       PSUM accumulate), chunked N<=512.
  ACT: evacuate vs/vi PSUM -> SBUF fp16.
  DVE: Ix = vs[c-1]-vs[c+1]; Iy = vi[c-1]+vi[c+1]+(10/3)*vi[c];
       products Pxx, Pyy, Pxy (fp16).
  PE : A = box2d(Pxx), B = box2d(Pyy), C' = 2*box2d(Pxy) via banded
       lhsT x 3 horizontally-shifted rhs, PSUM accumulate.
  ACT/DVE tail: trace=A+B, diff=A-B, E = diff^2 + C'^2, eig = trace -
       sqrt(E); edge = 0.5|Ix| + 0.5|Iy|; fp32 out, DMA to DRAM.

Image-boundary zero-pad semantics of the reference (products are
zero-padded before the box conv) are handled by per-core box-weight
variants (top/bottom row kill) and by zeroed border columns.
"""

import numpy as np

# ---------------------------------------------------------------------------
# BIR patch: this container's walrus build accepts only ONE sync-wait per
# instruction, but Tile's kernel-tail Drain aggregates one wait per logical
# processor.  Split any instruction carrying >1 waits into preceding
# same-engine Drain clones each carrying a single wait.
# ---------------------------------------------------------------------------
import orjson

_MAX_WAITS = 1


def _split_block(insts):
    out = []
    ctr = 0
    for inst in insts:
        si = inst.get("sync_info")
        ow = (si or {}).get("on_wait") or []
        if len(ow) > _MAX_WAITS:
            extra, keep = ow[:-_MAX_WAITS], ow[-_MAX_WAITS:]
            for i in range(0, len(extra), _MAX_WAITS):
                out.append(
                    {
                        "name": f"{inst['name']}-ws{ctr}",
                        "opcode": "Drain",
                        "engine": inst["engine"],
                        "ins": [],
                        "outs": [],
                        "is_reset_sema": False,
                        "debug": inst.get("debug", 0),
                        "sync_info": {
                            "on_update": [],
                            "on_wait": extra[i : i + _MAX_WAITS],
                        },
                    }
                )
                ctr += 1
            si["on_wait"] = keep
        out.append(inst)
    return out


def _split_sem_waits(bir_json: bytes) -> bytes:
    d = orjson.loads(bir_json)
    changed = False
    for fn in d.get("functions", []):
        for blk in fn.get("blocks", []):
            insts = blk.get("instructions", [])
            if any(
                len(((i.get("sync_info") or {}).get("on_wait") or [])) > _MAX_WAITS
                for i in insts
            ):
                blk["instructions"] = _split_block(insts)
                changed = True
    return orjson.dumps(d) if changed else bir_json


def _install_birpatch():
    import concourse.bass_utils as bu
    import concourse.bass2jax as b2j

    if getattr(bu.compile_bir_kernel, "_waitsplit", False):
        return
    orig = bu.compile_bir_kernel

    def patched(bir_json, tmpdir, neff_name="file.neff"):
        return orig(_split_sem_waits(bir_json), tmpdir, neff_name)

    patched._waitsplit = True
    bu.compile_bir_kernel = patched
    b2j.compile_bir_kernel = patched


_install_birpatch()

import concourse.bass as bass
import concourse.tile as tile
from concourse import mybir
from concourse.bass_utils import run_bass_kernel_spmd

# ---------------------------------------------------------------------------
# Geometry constants
# ---------------------------------------------------------------------------
N_CORES = 8
H = W = 4096
BAND = H // N_CORES          # 512 output rows per core
SLAB = 520                   # input rows per core: band + 2 halo top + 6 pad
PW = W + 8                   # host-padded width, image at cols [2, 4098)
NSTRIPE = 2
SW = 2056                    # stripe buffer width (2048 + 4 halo + 4 pad)
SIMG = 2048                  # image cols per stripe
TILE_T = [0, 97, 194, 291, 388]          # tile start row within slab
# per-tile store range in slab-row space [lo, hi)
STORE = [(2, 99), (99, 196), (196, 293), (293, 390), (390, 514)]
GRAD_CHUNKS = [(0, 512), (512, 1024), (1024, 1536), (1536, 2048), (2048, SW)]
BOX_CHUNKS = [(2, 514), (514, 1026), (1026, 1538), (1538, 2050)]

GRAY_W = np.array([0.2989, 0.587, 0.114], dtype=np.float32)
SMOOTH = np.array([3.0, 10.0, 3.0], dtype=np.float32) / 16.0
INTERP = np.array([1.0, 0.0, -1.0], dtype=np.float32)

F32 = mybir.dt.float32
F16 = mybir.dt.float16


def _band_lhsT(coeffs, scale=1.0, m_lo=0, m_hi=128, kill_rows=()):
    """lhsT[k, m] = coeffs[k - m + 1] * scale  for |k-m|<=1, m in [m_lo,m_hi).

    out[m] = sum_k lhsT[k, m] * rhs[k]  (vertical 3-tap conv over partitions).
    kill_rows: k indices to zero entirely (image-boundary pad semantics).
    """
    w = np.zeros((128, 128), dtype=np.float32)
    for m in range(m_lo, m_hi):
        for dk in (-1, 0, 1):
            k = m + dk
            if 0 <= k < 128:
                w[k, m] = coeffs[dk + 1] * scale
    for k in kill_rows:
        w[k, :] = 0.0
    return w


def _weights_for_core(core):
    """All [128,128] lhsT weight matrices for one core."""
    ws = {}
    for c in range(3):
        ws[f"wvs{c}"] = _band_lhsT(SMOOTH, scale=float(GRAY_W[c]))
        ws[f"wvi{c}"] = _band_lhsT(INTERP, scale=float(GRAY_W[c]) * 3.0 / 16.0)
    ones = np.array([1.0, 1.0, 1.0], dtype=np.float32)
    box_mid1 = _band_lhsT(ones, 1.0, 2, 126)
    box_mid2 = _band_lhsT(ones, 2.0, 2, 126)
    # tile 0 of core 0: slab partition 1 is image row -1 -> exclude from box
    k0 = (1,) if core == 0 else ()
    # tile 4 of core 7: slab partition 126 (row T=388 + 126 -> slab row 514)
    # is image row 4096 -> exclude from box
    k4 = (126,) if core == N_CORES - 1 else ()
    ws["wbox1_t0"] = _band_lhsT(ones, 1.0, 2, 126, k0)
    ws["wbox2_t0"] = _band_lhsT(ones, 2.0, 2, 126, k0)
    ws["wbox1_mid"] = box_mid1
    ws["wbox2_mid"] = box_mid2
    ws["wbox1_t4"] = _band_lhsT(ones, 1.0, 2, 126, k4)
    ws["wbox2_t4"] = _band_lhsT(ones, 2.0, 2, 126, k4)
    return {k: v.astype(np.float16 if k.startswith("wbox") else np.float32)
            for k, v in ws.items()}


WEIGHT_NAMES = [f"wvs{c}" for c in range(3)] + [f"wvi{c}" for c in range(3)] + [
    "wbox1_t0", "wbox2_t0", "wbox1_mid", "wbox2_mid", "wbox1_t4", "wbox2_t4",
]


# ---------------------------------------------------------------------------
# Kernel build
# ---------------------------------------------------------------------------
def build_nc():
    from contextlib import ExitStack

    nc = bass.Bass("TRN2", target_bir_lowering=False, num_devices=N_CORES)
    xs = nc.declare_dram_parameter("xs", [3, SLAB, PW], F32, isOutput=False)
    wt = {}
    for name in WEIGHT_NAMES:
        dt = F16 if name.startswith("wbox") else F32
        wt[name] = nc.declare_dram_parameter(name, [128, 128], dt, isOutput=False)
    edge_o = nc.declare_dram_parameter("edge", [BAND, W], F32, isOutput=True)
    eig_o = nc.declare_dram_parameter("eig", [BAND, W], F32, isOutput=True)

    with ExitStack() as ctx:
        tc = ctx.enter_context(tile.TileContext(nc))
        singles = ctx.enter_context(tc.tile_pool(name="singles", bufs=1))
        xpool = ctx.enter_context(tc.tile_pool(name="x", bufs=2))
        gsb = ctx.enter_context(tc.tile_pool(name="gsb", bufs=2))
        ixy = ctx.enter_context(tc.tile_pool(name="ixy", bufs=1))
        tmp = ctx.enter_context(tc.tile_pool(name="tmp", bufs=1))
        prod = ctx.enter_context(tc.tile_pool(name="prod", bufs=2))
        tailA = ctx.enter_context(tc.tile_pool(name="tailA", bufs=2))
        tailB = ctx.enter_context(tc.tile_pool(name="tailB", bufs=1))
        outp = ctx.enter_context(tc.tile_pool(name="outp", bufs=2))
        psg = ctx.enter_context(tc.tile_pool(name="psg", bufs=2, space="PSUM"))
        psb = ctx.enter_context(tc.tile_pool(name="psb", bufs=1, space="PSUM"))

        # weights resident in SBUF
        wsb = {}
        for name in WEIGHT_NAMES:
            t = singles.tile([128, 128], wt[name].dtype, name=name, tag=name)
            nc.sync.dma_start(out=t[:], in_=wt[name][:, :])
            wsb[name] = t

        for ti, T in enumerate(TILE_T):
            box1 = wsb["wbox1_t0" if ti == 0 else ("wbox1_t4" if ti == 4 else "wbox1_mid")]
            box2 = wsb["wbox2_t0" if ti == 0 else ("wbox2_t4" if ti == 4 else "wbox2_mid")]
            st_lo, st_hi = STORE[ti]
            p_lo, p_hi = st_lo - T, st_hi - T        # store partition range
            for s in range(NSTRIPE):
                col0 = SIMG * s                      # xs col of stripe buf col 0
                xt = [
                    xpool.tile([128, SW], F32, tag=f"x{c}", name=f"x{c}")
                    for c in range(3)
                ]
                for c in range(3):
                    nc.sync.dma_start(
                        out=xt[c][:],
                        in_=xs[c, T : T + 128, col0 : col0 + SW],
                    )

                vs_sb = gsb.tile([128, SW], F16, tag="vs")
                vi_sb = gsb.tile([128, SW], F16, tag="vi")
                for lo, hi in GRAD_CHUNKS:
                    n = hi - lo
                    vs_ps = psg.tile([128, 512], F32, tag="vs_ps")
                    vi_ps = psg.tile([128, 512], F32, tag="vi_ps")
                    for c in range(3):
                        nc.tensor.matmul(
                            vs_ps[:, :n], wsb[f"wvs{c}"][:], xt[c][:, lo:hi],
                            start=(c == 0), stop=(c == 2),
                        )
                    for c in range(3):
                        nc.tensor.matmul(
                            vi_ps[:, :n], wsb[f"wvi{c}"][:], xt[c][:, lo:hi],
                            start=(c == 0), stop=(c == 2),
                        )
                    nc.scalar.copy(out=vs_sb[:, lo:hi], in_=vs_ps[:, :n])
                    nc.scalar.copy(out=vi_sb[:, lo:hi], in_=vi_ps[:, :n])

                # gradients; writes cols [1, 2051)
                ix = ixy.tile([128, SW], F16, tag="ix")
                iy = ixy.tile([128, SW], F16, tag="iy")
                nc.vector.tensor_tensor(
                    ix[:, 1:2051], vs_sb[:, 0:2050], vs_sb[:, 2:2052],
                    mybir.AluOpType.subtract,
                )
                t2 = tmp.tile([128, SW], F16, tag="t2")
                u = tmp.tile([128, SW], F16, tag="u")
                nc.vector.tensor_tensor(
                    t2[:, 1:2051], vi_sb[:, 0:2050], vi_sb[:, 2:2052],
                    mybir.AluOpType.add,
                )
                nc.vector.tensor_scalar(
                    out=u[:, 1:2051], in0=vi_sb[:, 1:2051],
                    scalar1=float(10.0 / 3.0), scalar2=None,
                    op0=mybir.AluOpType.mult,
                )
                nc.vector.tensor_tensor(
                    iy[:, 1:2051], t2[:, 1:2051], u[:, 1:2051],
                    mybir.AluOpType.add,
                )

                # products, fp16; border cols zeroed for box zero-pad
                pxx = prod.tile([128, SW], F16, tag="pxx")
                pyy = prod.tile([128, SW], F16, tag="pyy")
                pxy = prod.tile([128, SW], F16, tag="pxy")
                plo, phi = (2, 2051) if s == 0 else (1, 2050)
                for p, a, b in ((pxx, ix, ix), (pyy, iy, iy), (pxy, ix, iy)):
                    if s == 0:
                        nc.vector.memset(p[:, 1:2], 0.0)
                    else:
                        nc.vector.memset(p[:, 2050:2051], 0.0)
                    nc.vector.tensor_tensor(
                        p[:, plo:phi], a[:, plo:phi], b[:, plo:phi],
                        mybir.AluOpType.mult,
                    )

                a_sb = tailA.tile([128, SW], F16, tag="a_sb")
                tr_sb = tailB.tile([128, SW], F16, tag="tr")
                df_sb = tailB.tile([128, SW], F16, tag="df")
                cq_sb = tailA.tile([128, SW], F16, tag="cq")
                for lo, hi in BOX_CHUNKS:
                    n = hi - lo
                    a_ps = psb.tile([128, 512], F32, tag="a_ps")
                    b_ps = psb.tile([128, 512], F32, tag="b_ps")
                    c_ps = psb.tile([128, 512], F32, tag="c_ps")
                    for di, d in enumerate((-1, 0, 1)):
                        nc.tensor.matmul(
                            a_ps[:, :n], box1[:], pxx[:, lo + d : hi + d],
                            start=(di == 0), stop=(di == 2),
                        )
                    for di, d in enumerate((-1, 0, 1)):
                        nc.tensor.matmul(
                            b_ps[:, :n], box1[:], pyy[:, lo + d : hi + d],
                            start=(di == 0), stop=(di == 2),
                        )
                    for di, d in enumerate((-1, 0, 1)):
                        nc.tensor.matmul(
                            c_ps[:, :n], box2[:], pxy[:, lo + d : hi + d],
                            start=(di == 0), stop=(di == 2),
                        )
                    nc.scalar.copy(out=a_sb[:, lo:hi], in_=a_ps[:, :n])
                    nc.vector.tensor_tensor(
                        tr_sb[:, lo:hi], b_ps[:, :n], a_sb[:, lo:hi],
                        mybir.AluOpType.add,
                    )
                    nc.vector.tensor_tensor(
                        df_sb[:, lo:hi], a_sb[:, lo:hi], b_ps[:, :n],
                        mybir.AluOpType.subtract,
                    )
                    nc.scalar.square(out=cq_sb[:, lo:hi], in_=c_ps[:, :n])

                dd = tailB.tile([128, SW], F16, tag="dd")
                ee = tailA.tile([128, SW], F16, tag="ee")
                ss = tailA.tile([128, SW], F16, tag="ss")
                nc.vector.tensor_tensor(
                    dd[:, 2:2050], df_sb[:, 2:2050], df_sb[:, 2:2050],
                    mybir.AluOpType.mult,
                )
                nc.vector.tensor_tensor(
                    ee[:, 2:2050], dd[:, 2:2050], cq_sb[:, 2:2050],
                    mybir.AluOpType.add,
                )
                nc.scalar.sqrt(out=ss[:, 2:2050], in_=ee[:, 2:2050])
                eig_sb = outp.tile([128, SW], F32, tag="eig")
                nc.vector.tensor_tensor(
                    eig_sb[:, 2:2050], tr_sb[:, 2:2050], ss[:, 2:2050],
                    mybir.AluOpType.subtract,
                )

                aix = tmp.tile([128, SW], F16, tag="aix")
                aiy = tmp.tile([128, SW], F16, tag="aiy")
                nc.scalar.activation(
                    out=aix[:, 2:2050], in_=ix[:, 2:2050],
                    func=mybir.ActivationFunctionType.Abs, scale=0.5,
                )
                nc.scalar.activation(
                    out=aiy[:, 2:2050], in_=iy[:, 2:2050],
                    func=mybir.ActivationFunctionType.Abs, scale=0.5,
                )
                edge_sb = outp.tile([128, SW], F32, tag="edge")
                nc.vector.tensor_tensor(
                    edge_sb[:, 2:2050], aix[:, 2:2050], aiy[:, 2:2050],
                    mybir.AluOpType.add,
                )

                # store: slab rows [st_lo, st_hi) -> out rows [st_lo-2, st_hi-2)
                nc.sync.dma_start(
                    out=edge_o[st_lo - 2 : st_hi - 2, SIMG * s : SIMG * (s + 1)],
                    in_=edge_sb[p_lo:p_hi, 2:2050],
                )
                nc.sync.dma_start(
                    out=eig_o[st_lo - 2 : st_hi - 2, SIMG * s : SIMG * (s + 1)],
                    in_=eig_sb[p_lo:p_hi, 2:2050],
                )
    return nc


_NC_CACHE = None


def _get_nc():
    global _NC_CACHE
    if _NC_CACHE is None:
        _NC_CACHE = build_nc()
    return _NC_CACHE


def kernel(x, edge_filter):
    x = np.asarray(x, dtype=np.float32)
    nc = _get_nc()

    gxp = np.zeros((3, H + 8, PW), dtype=np.float32)
    gxp[:, 2 : 2 + H, 2 : 2 + W] = x[0]

    in_maps = []
    for k in range(N_CORES):
        m = {"xs": np.ascontiguousarray(gxp[:, BAND * k : BAND * k + SLAB, :])}
        m.update(_weights_for_core(k))
        in_maps.append(m)

    res = run_bass_kernel_spmd(nc, in_maps, list(range(N_CORES)))
    edge = np.empty((1, H, W), dtype=np.float32)
    eig = np.empty((1, H, W), dtype=np.float32)
    for k in range(N_CORES):
        edge[0, BAND * k : BAND * (k + 1)] = res.results[k]["edge"]
        eig[0, BAND * k : BAND * (k + 1)] = res.results[k]["eig"]
    return (edge, eig)
